# revision 18
# baseline (speedup 1.0000x reference)
"""Trainium2 Bass kernel: pointwise-conv (GEMM) + BatchNorm (folded) + LIF scan
+ spike-rate mean, sharded over 8 NeuronCores by TIME chunks.

Reference semantics (fp32):
    y   = einsum('bct,oc->bot', x, W)                   # [B, Cout, T]
    yb  = (y - mean) * (rsqrt(var+eps) * gamma) + beta  # BN (inference)
    v' = v + (yb_t - v)/2 ; s = (v' >= 1) ; v'' = v' * (1-s)   # LIF, T steps
    out = mean_t(s)                                     # [B, Cout]

Key facts exploited:
  * BN + the 1/TAU charge factor fold into the conv weights on the host:
        z_t = (0.5*gamma*rsqrt(var+eps) * W) @ x_t + bias
    and the LIF step becomes  u = 0.5*v + z ; spike = u>=1 ; v = u*(u<1).
  * The LIF recurrence forgets its state at rate 0.5/step (hard reset only
    accelerates forgetting), so a time chunk can be computed exactly from a
    zero state started WARM steps earlier: state influence ~0.5^32 ~ 2e-10
    cannot flip an fp32 threshold comparison except for astronomically
    unlikely near-ties.  Time-sharding is thus embarrassingly parallel with
    a WARM-step overlap.  (Core 0's warmup input columns are zeroed so its
    state stays exactly 0.)
  * fp32 matmuls run at 1/4 PE rate; instead split both operands into
    bf16 hi+lo and take 3 bf16 matmuls (hi*hi + hi*lo + lo*hi), all at
    full PE rate with fp32 PSUM accumulation.  Residual ~2^-18 relative —
    empirically the same single spike-flip vs the jax reference as an
    exact fp32 matmul (the flip comes from summation-order rounding).
    Also halves DMA bytes (bf16 pairs = 4B per original fp32 element).
  * scalar_tensor_tensor (DVE/GPSIMD) computes (in0 op0 scalar) op1 in1 in
    ONE op:
        u   = (v  * 0.5) + z        (mult, add)    DVE
        v   = (u <  1.0) * u        (is_lt, mult)  DVE  <- hard reset fused
        acc = (u >= 1.0) + acc      (is_ge, add)   GPSIMD (off critical path)

Per core c (of 8): time window [128c - 32, 128c + 128); spikes counted only
for the core's own 128 steps; host sums the 8 partial counts / 1024.
"""

import sys
import numpy as np

if "/opt/trn_rl_repo" not in sys.path:
    sys.path.insert(0, "/opt/trn_rl_repo")

# --- problem constants (hardcoded; kernel.py must be self-contained) ---
B, CIN, T, COUT = 64, 512, 1024, 256
NCORES = 8
WARM = 32                    # warmup steps per core (exact: 0.5^32 decay)
TCH = T // NCORES            # 128 owned steps / core
TLOC = WARM + TCH            # 160 local steps
TS = 16                      # time-steps per matmul/psum window
NTS = TLOC // TS             # 10 windows
KO = CIN // 128              # 4 contraction chunks
NBB = 32                     # batches per matmul N-chunk (N = NBB*TS = 512)
BN_EPS = 1e-5

_CACHE = {}

# "bf16x3": 3 bf16 hi/lo-split matmuls at full PE rate (default)
# "f32"   : exact fp32 matmuls (4 cycles/row on PE)
MM_MODE = "bf16x3"
# engine that accumulates the spike count:
#   "vector"     - 3rd fused STT op per step on DVE
#   "pool_block" - per-16-step block on GPSIMD: is_ge mask + tree-add
#                  (keeps DVE at 2 ops/step; Pool is otherwise idle)
COUNT_ENGINE = "vector"


def _build_nc(with_bias: bool, mm_mode: str = MM_MODE,
              count_engine: str = COUNT_ENGINE, reps: int = 1,
              loop_reps: int = 0):
    import concourse.tile as tile
    from concourse import bacc, mybir

    f32 = mybir.dt.float32
    bf16 = mybir.dt.bfloat16
    op = mybir.AluOpType
    split = mm_mode == "bf16x3"
    x_dt = bf16 if split else f32
    nhl = 2 if split else 1

    nc = bacc.Bacc(None)
    # per-core inputs, host-prearranged so every DMA is one contiguous block:
    #   xk [KO, 128, NTS, nhl, B, TS]  (nhl=2: bf16 hi/lo split of x)
    #   wT [nhl, CIN, COUT]            (folded weights, k-major)
    xk = nc.declare_dram_parameter("xk", [KO, 128, NTS, nhl, B, TS], x_dt,
                                   isOutput=False)
    wT = nc.declare_dram_parameter("wT", [nhl, CIN, COUT], x_dt, isOutput=False)
    if with_bias:
        bvec = nc.declare_dram_parameter("bvec", [1, 2, 128], f32, isOutput=False)
    counts = nc.declare_dram_parameter("counts", [128, 2, B], f32, isOutput=True)

    with tile.TileContext(nc) as tc:
        with (
            tc.tile_pool(name="consts", bufs=1) as consts,
            tc.tile_pool(name="xs", bufs=2) as xs,
            tc.tile_pool(name="zs", bufs=3) as zs,
            tc.tile_pool(name="psum", bufs=2, space="PSUM") as psum,
        ):
            # folded weights: [ki, hl, ko, m] with m = ch*128 + mi
            w_sb = consts.tile([128, nhl, KO, COUT], x_dt)
            nc.sync.dma_start(
                w_sb, wT.rearrange("h (ko ki) m -> ki h ko m", ki=128))

            bias_sb = ones_sb = None
            if with_bias:
                bias_sb = consts.tile([1, 2, 128], f32)
                nc.sync.dma_start(bias_sb, bvec[:])
                ones_sb = consts.tile([1, NBB * TS], f32)
                nc.vector.memset(ones_sb, 1.0)

            v = consts.tile([128, 2, B], f32)
            acc = consts.tile([128, 2, B], f32)

            # reps>1 / loop_reps>0 repeat the compute for benchmarking only
            if loop_reps > 0:
                with tc.For_i(0, loop_reps, 1):
                    _emit_body(nc, tc, xs, zs, psum, xk, counts, w_sb, v, acc,
                               bias_sb, ones_sb, split, count_engine, op, f32,
                               x_dt, mybir)
            else:
                for _rep in range(reps):
                    _emit_body(nc, tc, xs, zs, psum, xk, counts, w_sb, v, acc,
                               bias_sb, ones_sb, split, count_engine, op, f32,
                               x_dt, mybir)

    if not nc.is_finalized():
        nc.finalize()
    return nc


def _emit_body(nc, tc, xs, zs, psum, xk, counts, w_sb, v, acc,
               bias_sb, ones_sb, split, count_engine, op, f32, x_dt, mybir):
    with_bias = bias_sb is not None
    nhl = 2 if split else 1
    # (w_half, x_half) term list: hi*hi + hi*lo + lo*hi
    terms = [(0, 0), (0, 1), (1, 0)] if split else [(0, 0)]

    nc.vector.memset(v, 0.0)
    nc.vector.memset(acc, 0.0)

    for tsi in range(NTS):
        # ---- load x window: KO tiles of [128, nhl, B, TS] (contig) ----
        xts = []
        for ko in range(KO):
            xt = xs.tile([128, nhl, B, TS], x_dt, tag=f"x{ko}")
            nc.sync.dma_start(xt, xk[ko, :, tsi])
            xts.append(xt)

        # ---- matmul: psum[:, ch, (b,t)] += W'.T @ x  (split terms) ----
        pt = psum.tile([128, 2, B * TS], f32)
        for ch in range(2):
            n_acc = len(terms) * KO
            i_acc = 0
            for ko in range(KO):
                for (wh, xh) in terms:
                    lhsT = w_sb[:, wh, ko, ch * 128:(ch + 1) * 128]
                    first = i_acc == 0
                    last = i_acc == n_acc - 1
                    i_acc += 1
                    for nb in range(B // NBB):
                        nc.tensor.matmul(
                            pt[:, ch, nb * NBB * TS:(nb + 1) * NBB * TS],
                            lhsT,
                            xts[ko][:, xh, nb * NBB:(nb + 1) * NBB, :],
                            start=first,
                            stop=(last and not with_bias),
                        )
            if with_bias:
                for nb in range(B // NBB):
                    nc.tensor.matmul(
                        pt[:, ch, nb * NBB * TS:(nb + 1) * NBB * TS],
                        bias_sb[:, ch, :],
                        ones_sb,
                        start=False,
                        stop=True,
                    )

        # ---- evacuate psum -> sbuf z-block [128, TS, 2, B] (ACT) ----
        zb = zs.tile([128, TS, 2, B], f32, tag="zb")
        for ch in range(2):
            nc.scalar.copy(
                out=zb[:, :, ch, :],
                in_=pt[:, ch].rearrange("p (b t) -> p t b", t=TS),
            )

        # ---- LIF scan: 2 fused DVE ops/step (+ count) ----
        for ti in range(TS):
            t = tsi * TS + ti
            u = zb[:, ti]  # holds z_t; overwritten in place with u_t
            nc.vector.scalar_tensor_tensor(
                out=u, in0=v, scalar=0.5, in1=u,
                op0=op.mult, op1=op.add,
            )
            if t >= WARM and count_engine == "vector":
                nc.vector.scalar_tensor_tensor(
                    out=acc, in0=u, scalar=1.0, in1=acc,
                    op0=op.is_ge, op1=op.add,
                )
            nc.vector.scalar_tensor_tensor(
                out=v, in0=u, scalar=1.0, in1=u,
                op0=op.is_lt, op1=op.mult,
            )

        if count_engine == "pool_block" and tsi * TS >= WARM:
            # zb still holds all 16 u_t tiles; count spikes on GPSIMD
            mblk = zs.tile([128, TS, 2, B], f32, tag="mblk")
            nc.gpsimd.tensor_scalar(
                out=mblk[:], in0=zb[:], scalar1=1.0, scalar2=None,
                op0=op.is_ge,
            )
            h = TS
            while h > 1:
                h //= 2
                nc.gpsimd.tensor_tensor(
                    out=mblk[:, :h], in0=mblk[:, :h], in1=mblk[:, h:2 * h],
                    op=op.add,
                )
            nc.gpsimd.tensor_tensor(
                out=acc, in0=acc, in1=mblk[:, 0], op=op.add,
            )

    nc.sync.dma_start(counts[:], acc)


def _split_bf16(a):
    """fp32 -> (hi, lo) bf16 pair with hi + lo ~ a (error ~2^-18 relative)."""
    import ml_dtypes
    hi = a.astype(ml_dtypes.bfloat16)
    lo = (a - hi.astype(np.float32)).astype(ml_dtypes.bfloat16)
    return hi, lo


def _prep_inputs(x, W, gamma, beta, run_mean, run_var, mm_mode=None):
    """Fold BN + 1/TAU into weights; build per-core time-sharded x layouts."""
    if mm_mode is None:
        mm_mode = MM_MODE
    split = mm_mode == "bf16x3"
    import ml_dtypes

    inv = 1.0 / np.sqrt(run_var.astype(np.float64) + BN_EPS)
    a = (0.5 * gamma.astype(np.float64) * inv)
    Wp = (W.astype(np.float64) * a[:, None]).astype(np.float32)       # [COUT, CIN]
    bp = (0.5 * (beta.astype(np.float64)
                 - run_mean.astype(np.float64) * gamma.astype(np.float64) * inv)
          ).astype(np.float32)                                        # [COUT]
    wT = np.ascontiguousarray(Wp.T)                                   # [CIN, COUT]
    if split:
        wh, wl = _split_bf16(wT)
        wTs = np.ascontiguousarray(np.stack([wh, wl], axis=0))        # [2,CIN,COUT]
        xh, xl = _split_bf16(x)
        xhl = np.stack([xh, xl], axis=0)                              # [2,B,CIN,T]
    else:
        wTs = wT.reshape(1, CIN, COUT)

    in_maps = []
    for c in range(NCORES):
        t0 = c * TCH - WARM
        lo = max(t0, 0)
        if split:
            xc = np.zeros((2, B, CIN, TLOC), dtype=ml_dtypes.bfloat16)
            xc[:, :, :, lo - t0:] = xhl[:, :, :, lo:c * TCH + TCH]
            # [2, B, CIN, TLOC] -> [KO, 128, NTS, 2, B, TS]
            xkc = np.ascontiguousarray(
                xc.reshape(2, B, KO, 128, NTS, TS).transpose(2, 3, 4, 0, 1, 5)
            )
        else:
            xc = np.zeros((B, CIN, TLOC), dtype=np.float32)
            xc[:, :, lo - t0:] = x[:, :, lo:c * TCH + TCH]
            xkc = np.ascontiguousarray(
                xc.reshape(B, KO, 128, NTS, TS).transpose(1, 2, 3, 0, 4)
            )[:, :, :, None]
        m = {"xk": xkc, "wT": wTs}
        if np.any(bp != 0):
            m["bvec"] = np.ascontiguousarray(bp.reshape(1, 2, 128))
        in_maps.append(m)
    return in_maps, bool(np.any(bp != 0))


def _postprocess(results):
    total = np.zeros((128, 2, B), dtype=np.float64)
    for r in results:
        total += r["counts"].astype(np.float64)
    # counts[ci, ch, b] -> out[b, ch*128+ci]
    out = total.transpose(2, 1, 0).reshape(B, COUT) / float(T)
    return out.astype(np.float32)


def kernel(x, W, gamma, beta, run_mean, run_var, _trace=False):
    from concourse.bass_utils import run_bass_kernel_spmd

    x = np.asarray(x, dtype=np.float32)
    W = np.asarray(W, dtype=np.float32)
    gamma = np.asarray(gamma, dtype=np.float32)
    beta = np.asarray(beta, dtype=np.float32)
    run_mean = np.asarray(run_mean, dtype=np.float32)
    run_var = np.asarray(run_var, dtype=np.float32)

    in_maps, with_bias = _prep_inputs(x, W, gamma, beta, run_mean, run_var)
    key = ("nc", with_bias, MM_MODE, COUNT_ENGINE)
    if key not in _CACHE:
        _CACHE[key] = _build_nc(with_bias)
    nc = _CACHE[key]

    res = run_bass_kernel_spmd(
        nc, in_maps, core_ids=list(range(NCORES)), trace=_trace
    )
    out = _postprocess(res.results)
    if _trace:
        return out, res
    return out


if __name__ == "__main__":
    rng = np.random.default_rng(0)
    x = rng.standard_normal((B, CIN, T), dtype=np.float32)
    W = (rng.standard_normal((COUT, CIN), dtype=np.float32) / np.sqrt(CIN)).astype(np.float32)
    out = kernel(x, W, np.ones(COUT, np.float32), np.zeros(COUT, np.float32),
                 np.zeros(COUT, np.float32), np.ones(COUT, np.float32))
    print(out.shape, out.dtype, out[:2, :4])
